# revision 10
# baseline (speedup 1.0000x reference)
"""ChannelShift kernel for Trainium2 (Bass), data-parallel over 8 NeuronCores.

Reference op (per sample, x viewed as [C, H*W] row-major):
  cols [0, FOLD)       : out[t] = x[t+1]  (zero at t=C-1)   -- shift left
  cols [FOLD, 2*FOLD)  : out[t] = x[t-1]  (zero at t=0)     -- shift right
  cols [2*FOLD, HW)    : out[t] = x[t]                       -- identity

Pure data movement: 5 strided DRAM->DRAM DMA copies (flat-row trick, see
_build_nc) that saturate all 16 SDMA engines, then 2 tiny zero-fill DMAs
(from a const-zero tensor baked into the NEFF) that fix up the per-sample
boundary rows after a semaphore wait.

Sharding: batch 64 -> 8 samples per core, no cross-core communication.
Measured ~172-200 us HW exec per core (run-to-run tunnel variance), vs
~157 us all-engines-busy floor at the observed ~20.5 GB/s per-engine
HBM<->HBM DMA rate.
"""

import numpy as np

import concourse.bass as bass
import concourse.mybir as mybir
from concourse.bass_utils import run_bass_kernel_spmd

BS, C, H, W = 64, 512, 56, 56
HW = H * W              # 3136
FOLD = HW // 8          # 392
N_CORES = 8
BS_PER = BS // N_CORES  # 8

_nc_cache = None


def _build_nc() -> bass.Bass:
    nc = bass.Bass()
    x = nc.declare_dram_parameter("x", [BS_PER, C, HW], mybir.dt.float32, isOutput=False)
    out = nc.declare_dram_parameter("out", [BS_PER, C, HW], mybir.dt.float32, isOutput=True)
    zeros = nc.inline_tensor(np.zeros((BS_PER, FOLD), np.float32), name="zeros")

    with (
        nc.Block() as block,
        nc.semaphore("dma_sem") as dma_sem,
        nc.semaphore("id_sem") as id_sem,
    ):

        @block.scalar
        def _(scalar):
            # Identity band (75% of bytes) on the scalar HWDGE ring: its DGE
            # generates descriptors in parallel with the sync ring's (halving
            # the start ramp), and the zero fix-up below only has to wait for
            # the sync-ring band copies, hiding entirely under this stream.
            scalar.dma_start(
                out=out[:, :, 2 * FOLD :], in_=x[:, :, 2 * FOLD :]
            ).then_inc(id_sem, 16)

        @block.sync
        def _(sync):
            # The HWDGE splits one HBM->HBM DMA across ndma engines where ndma
            # is the largest n<=16 dividing the OUTERMOST AP dim count
            # (dge_reshape.cpp: choose_tensor_reshape_kind_none_special), and
            # each engine gets a CONTIGUOUS chunk of the outer dim - so the
            # outer count must divide by 16 AND consecutive descriptors should
            # be address-sequential for HBM locality (sample-interleaved order
            # measured 2x slower per engine).
            #
            # Trick: treat (bs, C) as one flat row axis R = bs*C. Within a
            # band, out_flat[r] = x_flat[r +/- 1] - garbage lands in the 16
            # per-sample boundary rows (overwritten with zeros afterwards,
            # order enforced by the sem wait). Flat row count splits as
            # 4080 (=16*255, 16 engines) + 15.
            n = 0
            R = BS_PER * C  # 4096 flat rows
            M = R - 16      # 4080, the 16-way-splittable main chunk
            F = FOLD
            xf = x.rearrange("s c p -> (s c) p")
            of = out.rearrange("s c p -> (s c) p")

            def dma(o, i):
                nonlocal n
                sync.dma_start(out=o, in_=i).then_inc(dma_sem, 16)
                n += 16

            # shift left band: out_flat[r, 0:F] = x_flat[r+1, 0:F], r in [0, R-1)
            dma(of[0:M, 0:F], xf[1 : M + 1, 0:F])
            dma(of[M : R - 1, 0:F], xf[M + 1 : R, 0:F])
            # shift right band: out_flat[r, F:2F] = x_flat[r-1, F:2F], r in [1, R)
            dma(of[1 : M + 1, F : 2 * F], xf[0:M, F : 2 * F])
            dma(of[M + 1 : R, F : 2 * F], xf[M : R - 1, F : 2 * F])
            # boundary rows get flat-copy garbage; zero them once the band
            # copies are done (identity writes disjoint columns - no wait)
            sync.wait_ge(dma_sem, n)
            dma(out[:, C - 1, 0:F], zeros[:, :])
            dma(out[:, 0, F : 2 * F], zeros[:, :])
            sync.wait_ge(dma_sem, n)
            sync.wait_ge(id_sem, 16)

    return nc


def _run(x: np.ndarray, trace: bool = False):
    """Shard, execute on 8 cores, return (full_output, BassKernelResults)."""
    global _nc_cache
    if _nc_cache is None:
        _nc_cache = _build_nc()
    nc = _nc_cache

    x3 = np.ascontiguousarray(np.asarray(x, dtype=np.float32).reshape(BS, C, HW))
    in_maps = [
        {"x": x3[i * BS_PER : (i + 1) * BS_PER]} for i in range(N_CORES)
    ]
    try:
        res = run_bass_kernel_spmd(nc, in_maps, list(range(N_CORES)), trace=trace)
    except Exception:
        # the axon tunnel occasionally throws a transient INTERNAL error;
        # one retry has been sufficient in practice
        res = run_bass_kernel_spmd(nc, in_maps, list(range(N_CORES)), trace=trace)
    out = np.concatenate([r["out"] for r in res.results], axis=0)
    return out.reshape(BS, C, H, W), res


def kernel(x: np.ndarray) -> np.ndarray:
    out, _ = _run(x, trace=False)
    return out
